# revision 1
# baseline (speedup 1.0000x reference)
"""Trainium2 Bass kernel for nn_BoundaryLoss (boundary-weighted NLL loss).

Contract: kernel(**inputs) takes FULL inputs (logits (8,20,512,512) f32,
targets (8,512,512) int), returns the FULL output (scalar f32 mean loss).
Internally shards batch across 8 NeuronCores (1 image per core), runs an
SPMD Bass program, and reduces the per-core partial sums on the host.

Math per image:
  boundaries = max_c sobel_mag(onehot(targets))   (3x3, replicate pad)
  weight = exp(clip(3*boundaries, 0, 5))
  nll    = logsumexp_c(logits) - logits[targets]
  out    = mean(weight * nll)

Key facts exploited:
  * sobel gradients per class are integers; mag^2 = X^2+Y^2 in {0,1,2,4,..}
    and weight only depends on min(mag^2, 3) in {0,1,2,3} -> exact cubic
    through {1, e^3, e^(3*sqrt(2)), e^5}: no sqrt/exp needed.
  * max over classes == max over the 8 neighbors' classes; X/Y for a
    neighbor's class are +/-1-weighted sums of 28 pairwise equality maps
    of shifted targets (all anchored at the output pixel).
  * logsumexp needs no max-subtraction for N(0,1) logits.
  * S2 = sum_c exp(x_c)*[c==t] = exp(x_t) -> nll = log(S1) - log(S2);
    sums over classes run on the TensorEngine with class-on-partition
    layout (6 groups x 20 classes = 120 partitions).
"""

import math
import os
import sys

import numpy as np

sys.path.insert(0, "/opt/trn_rl_repo")

import concourse.bass as bass  # noqa: E402
import concourse.tile as tile  # noqa: E402
from concourse import bacc, mybir  # noqa: E402
from concourse.bass_utils import run_bass_kernel_spmd  # noqa: E402

FP32 = mybir.dt.float32
BF16 = mybir.dt.bfloat16
I32 = mybir.dt.int32
Alu = mybir.AluOpType
Act = mybir.ActivationFunctionType

H = W = 512
C = 20
HW = H * W
B = 8
NCORES = 8
NRB = H // 128          # row blocks
NG = 6                  # class-layout groups
FG = 43691              # pixels per group (group 5 is 2 pixels short; its
                        # instructions run full width into padding)
HWP = HW + 2            # padded scratch row (absorbs group-5 tail garbage)
PADIN = 8               # host-side padding of flat inputs
SM_CHUNK = 512 * int(os.environ.get("KSMC3", "3"))  # softmax pixel chunk
MMF = 512               # matmul free width
CW = int(os.environ.get("KCW", "128"))  # boundary column chunk

# neighbor table: name -> (dy, dx, wx, wy)  (correlation w/ sobel kernels)
NEI = {
    "NW": (-1, -1, -1, -1), "N": (-1, 0, 0, -2), "NE": (-1, 1, 1, -1),
    "W": (0, -1, -2, 0), "E": (0, 1, 2, 0),
    "SW": (1, -1, -1, 1), "S": (1, 0, 0, 2), "SE": (1, 1, 1, 1),
}
KS = list(NEI)

# weight cubic through m3 in {0,1,2,3} -> {1, e^3, e^(3*sqrt2), e^5}
_WV = [1.0, math.exp(3.0), math.exp(3.0 * math.sqrt(2.0)), math.exp(5.0)]
_VAND = np.array([[m ** p for p in range(4)] for m in range(4)], np.float64)
PC = np.linalg.solve(_VAND, np.array(_WV, np.float64))  # c0..c3


def _pair_key(a, b):
    return tuple(sorted((a, b)))


def host_consts():
    import ml_dtypes
    c_col = (np.arange(120, dtype=np.float32) % 20).reshape(120, 1)
    lhs_rep = np.zeros((NG, 120), ml_dtypes.bfloat16)
    lhs_s1 = np.zeros((120, 32), ml_dtypes.bfloat16)
    lhs_s2 = np.zeros((120, 32), ml_dtypes.bfloat16)
    for g in range(NG):
        lhs_rep[g, 20 * g:20 * (g + 1)] = 1
        lhs_s1[20 * g:20 * (g + 1), g] = 1
        lhs_s2[20 * g:20 * (g + 1), 6 + g] = 1
    return {"c_col": c_col, "lhs_rep": lhs_rep,
            "lhs_s1": lhs_s1, "lhs_s2": lhs_s2}


def build_nc():
    nc = bacc.Bacc("TRN2", target_bir_lowering=False, debug=False)
    logits = nc.dram_tensor("logits", [C * HW + PADIN], FP32,
                            kind="ExternalInput")
    targets = nc.dram_tensor("targets", [H, W], I32, kind="ExternalInput")
    consts = {
        "c_col": nc.dram_tensor("c_col", [120, 1], FP32,
                                kind="ExternalInput"),
        "lhs_rep": nc.dram_tensor("lhs_rep", [NG, 120], BF16,
                                  kind="ExternalInput"),
        "lhs_s1": nc.dram_tensor("lhs_s1", [120, 32], BF16,
                                 kind="ExternalInput"),
        "lhs_s2": nc.dram_tensor("lhs_s2", [120, 32], BF16,
                                 kind="ExternalInput"),
    }
    out_partial = nc.dram_tensor("out_partial", [128, 1], FP32,
                                 kind="ExternalOutput")
    tpad = nc.dram_tensor("tpad", [H + 2, W + 8], BF16)   # row i-1, col j-4
    tflat = nc.dram_tensor("tflat", [HW + PADIN], BF16)
    s_scr = nc.dram_tensor("s_scr", [2, HWP], FP32)

    nrep = int(os.environ.get("KREPEAT", "1"))
    with tile.TileContext(nc) as tc:
        for _ in range(nrep):
            _body(tc, nc, logits, targets, out_partial, tpad, tflat, s_scr,
                  consts)
    nc.compile()
    return nc


def _body(tc, nc, logits, targets, out_partial, tpad, tflat, s_scr,
          consts):
    import contextlib
    ctx = contextlib.ExitStack()
    pool = ctx.enter_context(tc.tile_pool(name="main", bufs=1))
    tmp = ctx.enter_context(tc.tile_pool(name="tmp", bufs=8))
    psum = ctx.enter_context(
        tc.tile_pool(name="psum", bufs=1, space="PSUM"))

    # ---------------- phase 0: target prep + consts ----------------
    t_i32 = pool.tile([128, NRB, W], I32, tag="tI32")
    nc.sync.dma_start(
        t_i32[:], targets.ap().rearrange("(g p) w -> p g w", p=128))
    t_bf = pool.tile([128, NRB, W], BF16, tag="tBF")
    nc.vector.tensor_copy(t_bf[:], t_i32[:])

    nc.sync.dma_start(
        bass.AP(tensor=tflat, offset=0, ap=[[512, 128], [65536, NRB],
                                            [1, W]]),
        t_bf[:])
    # initialize the padding tail (values are never used meaningfully)
    nc.sync.dma_start(bass.AP(tensor=tflat, offset=HW, ap=[[1, PADIN]]),
                      t_bf[0:1, 0, 0:PADIN])

    # tpad interior
    nc.sync.dma_start(
        tpad.ap()[1:513, 4:516].rearrange("(g p) w -> p g w", p=128),
        t_bf[:])
    # left / right edge columns (replicate col 0 / col 511)
    for j in range(4):
        nc.sync.dma_start(
            tpad.ap()[1:513, j:j + 1].rearrange("(g p) w -> p g w", p=128),
            t_bf[:, :, 0:1])
        nc.sync.dma_start(
            tpad.ap()[1:513, 516 + j:517 + j].rearrange(
                "(g p) w -> p g w", p=128),
            t_bf[:, :, 511:512])
    # top / bottom replicate rows (row -1 = row 0, row 512 = row 511),
    # including corner padding
    nc.sync.dma_start(tpad.ap()[0:1, 4:516], t_bf[0:1, 0, :])
    nc.sync.dma_start(tpad.ap()[513:514, 4:516], t_bf[127:128, NRB - 1, :])
    for j in range(4):
        nc.sync.dma_start(tpad.ap()[0:1, j:j + 1], t_bf[0:1, 0, 0:1])
        nc.sync.dma_start(tpad.ap()[0:1, 516 + j:517 + j],
                          t_bf[0:1, 0, 511:512])
        nc.sync.dma_start(tpad.ap()[513:514, j:j + 1],
                          t_bf[127:128, NRB - 1, 0:1])
        nc.sync.dma_start(tpad.ap()[513:514, 516 + j:517 + j],
                          t_bf[127:128, NRB - 1, 511:512])

    # consts fed from host: c_col (class index per partition), mm weights
    c_col = pool.tile([120, 1], FP32, tag="cCol")
    nc.sync.dma_start(c_col[:], consts["c_col"].ap())
    lhs_rep = pool.tile([NG, 120], BF16, tag="lhsRep")
    nc.sync.dma_start(lhs_rep[:], consts["lhs_rep"].ap())
    lhs_s1 = pool.tile([120, 32], BF16, tag="lhsS1")
    nc.sync.dma_start(lhs_s1[:], consts["lhs_s1"].ap())
    lhs_s2 = pool.tile([120, 32], BF16, tag="lhsS2")
    nc.sync.dma_start(lhs_s2[:], consts["lhs_s2"].ap())

    PH = os.environ.get("KPHASES", "0ABCR")
    # ---------------- phase A: softmax sums (class-on-partition) -----
    # per chunk: x [120,CF] f32 ; exp bf16 ; t6 [6,CF] ; t_rep via PE ;
    # oh = (t_rep == c_col) ; expoh = exp*oh ; S1/S2 via PE -> psum ->
    # DRAM scratch (flat pixel-major).
    n_oh = 0
    for cf0 in (range(0, FG, SM_CHUNK) if "A" in PH else []):
        cf = min(SM_CHUNK, FG - cf0)
        xbufs = int(os.environ.get("KXBUFS", "2"))
        x_ck = pool.tile([120, cf], FP32, tag="xck", bufs=xbufs)
        nc.sync.dma_start(x_ck[:],
                          bass.AP(tensor=logits, offset=cf0,
                                  ap=[[FG, NG], [HW, C], [1, cf]]))

        e_ck = pool.tile([120, cf], BF16, tag="eck", bufs=2)
        nc.scalar.activation(e_ck[:], x_ck[:], Act.Exp)

        t6 = pool.tile([NG, cf], BF16, tag="t6", bufs=3)
        nc.scalar.dma_start(t6[:], bass.AP(tensor=tflat, offset=cf0,
                                           ap=[[FG, NG], [1, cf]]))

        oh_ck = pool.tile([120, cf], BF16, tag="ohck", bufs=2)
        for m0 in range(0, cf, MMF):
            mf = min(MMF, cf - m0)
            trep = psum.tile([120, mf], FP32, tag="trep", bufs=2)
            nc.tensor.matmul(trep[:], lhs_rep[:], t6[:, m0:m0 + mf])
            # one-hot: (t_rep == class(partition)); PSUM src -> DVE only
            nc.vector.tensor_scalar(oh_ck[:, m0:m0 + mf], trep[:],
                                    c_col[:], None, Alu.is_equal)
            n_oh += 1

        eo_ck = pool.tile([120, cf], BF16, tag="eock", bufs=2)
        eom = os.environ.get("KEOMUL", "alt")
        if eom == "gp" or (eom == "alt" and (cf0 // SM_CHUNK) % 2 == 0):
            eng_mul = nc.gpsimd
        else:
            eng_mul = nc.vector
        eng_mul.tensor_mul(eo_ck[:], e_ck[:], oh_ck[:])

        # S1/S2 sums: slice s of the chunk lands in psum rows 32s..32s+11
        # (matmul out base partition must be 0/32/64), 6 groups of S1 then
        # 6 of S2. Evacuation copy then runs at decent utilization.
        ns = (cf + MMF - 1) // MMF
        s12 = psum.tile([96, MMF], FP32, tag="s12", bufs=2)
        for s in range(ns):
            m0 = s * MMF
            mf = min(MMF, cf - m0)
            nc.tensor.matmul(s12[32 * s:32 * s + 32, 0:mf], lhs_s1[:],
                             e_ck[:, m0:m0 + mf], start=True, stop=False)
            nc.tensor.matmul(s12[32 * s:32 * s + 32, 0:mf], lhs_s2[:],
                             eo_ck[:, m0:m0 + mf], start=False, stop=True)
        s_sb = pool.tile([96, MMF], FP32, tag="ssb", bufs=2)
        use_act = (cf0 // SM_CHUNK) % 2 == 0
        def _cp(dst, src_):
            if use_act:
                nc.scalar.copy(dst, src_)
            else:
                nc.vector.tensor_copy(dst, src_)
        if cf == ns * MMF:
            _cp(s_sb[0:32 * ns, :], s12[0:32 * ns, :])
        else:
            _cp(s_sb[0:32 * (ns - 1), :], s12[0:32 * (ns - 1), :])
            mf_l = cf - (ns - 1) * MMF
            _cp(s_sb[32 * (ns - 1):32 * ns, 0:mf_l],
                s12[32 * (ns - 1):32 * ns, 0:mf_l])
        # SBUF -> DRAM scratch, flat pixel-major with uniform group
        # stride (psum rows per slice are field-major: 6*fld + g)
        for s in range(ns):
            m0 = s * MMF
            mf = min(MMF, cf - m0)
            dst = bass.AP(tensor=s_scr, offset=cf0 + m0,
                          ap=[[HWP, 2], [FG, NG], [1, mf]])
            eng_ev = nc.scalar if s % 2 == 0 else nc.sync
            eng_ev.dma_start(dst, s_sb[32 * s:32 * s + 12, 0:mf])

    # ---------------- phase B: boundary weights ----------------
    # stacks: rows shifted by r in {-1,0,1}; parityA cols -4.., parityB -3..
    stacks = {}
    for r in ((-1, 0, 1) if "B" in PH else []):
        sa = pool.tile([128, NRB, 520], BF16, tag=f"stA{r}")
        nc.sync.dma_start(
            sa[:],
            tpad.ap()[r + 1:r + 513, :].rearrange("(g p) c -> p g c", p=128))
        sb = pool.tile([128, NRB, 519], BF16, tag=f"stB{r}")
        nc.sync.dma_start(
            sb[:],
            tpad.ap()[r + 1:r + 513, 1:520].rearrange(
                "(g p) c -> p g c", p=128))
        stacks[r] = (sa, sb)

    def stack_ap(k, c0, cw):
        dy, dx, _, _ = NEI[k]
        sa, sb = stacks[dy]
        if (dx + c0) % 2 == 0:
            off = 4 + dx + c0
            return sa[:, :, off:off + cw]
        off = 3 + dx + c0
        return sb[:, :, off:off + cw]

    m3 = pool.tile([128, NRB, W], BF16, tag="m3")
    if "B" not in PH:
        nc.vector.memset(m3[:], 3.0)

    n_tt = [0]
    GP_OK = {Alu.add, Alu.subtract, Alu.mult}
    gprot = int(os.environ.get("KGPROT", "8"))

    def eng_tt(op=None):
        # distribute tensor-tensor work: mostly DVE, some gpsimd
        if op is not None and op not in GP_OK:
            return nc.vector
        n_tt[0] += 1
        return nc.gpsimd if (gprot and n_tt[0] % gprot == 0) else nc.vector

    for c0 in (range(0, W, CW) if "B" in PH else []):
        maps = {}
        todo = []
        for i, a in enumerate(KS):
            for b_ in KS[i + 1:]:
                todo.append(_pair_key(a, b_))
        for a, b_ in todo:
            mp = tmp.tile([128, NRB, CW], BF16, tag=f"map{a}{b_}", bufs=1)
            eng_tt(Alu.is_equal).tensor_tensor(
                mp[:], stack_ap(a, c0, CW), stack_ap(b_, c0, CW),
                Alu.is_equal)
            maps[(a, b_)] = mp

        def emap(a, b_):
            if a == b_:
                return None
            return maps[_pair_key(a, b_)]

        def tnew(tag):
            return tmp.tile([128, NRB, CW], BF16, tag=tag, bufs=2,
                            name=tag)

        def signed_diff(k, lp, lm, tag):
            """(tile, const) ~ e[k,lp] - e[k,lm], self maps -> const."""
            tp, tm = emap(k, lp), emap(k, lm)
            if tp is None:                      # 1 - e[k,lm]
                t = tnew(tag)
                nc.vector.tensor_scalar(t[:], tm[:], -1.0, 1.0,
                                        Alu.mult, Alu.add)
                return t, 0.0
            if tm is None:                      # e[k,lp] - 1
                return tp, -1.0
            t = tnew(tag)
            eng_tt(Alu.subtract).tensor_tensor(t[:], tp[:], tm[:],
                                               Alu.subtract)
            return t, 0.0

        mrun = None
        for k in KS:
            # corner diffs: P = e[k,SE]-e[k,NW], Q = e[k,NE]-e[k,SW]
            tP, cP = signed_diff(k, "SE", "NW", "cdP")
            tQ, cQ = signed_diff(k, "NE", "SW", "cdQ")
            # X = (P+Q) + 2*(e[k,E]-e[k,W]);  Y = (P-Q) + 2*(e[k,S]-e[k,N])
            tE, cE = signed_diff(k, "E", "W", "cdE")
            tS, cS = signed_diff(k, "S", "N", "cdS")
            pq_s = tnew("pqs")
            eng_tt(Alu.add).tensor_tensor(pq_s[:], tP[:], tQ[:], Alu.add)
            pq_d = tnew("pqd")
            eng_tt(Alu.subtract).tensor_tensor(pq_d[:], tP[:], tQ[:],
                                          Alu.subtract)
            ex2 = tnew("ex2")
            nc.vector.tensor_scalar(ex2[:], tE[:], 2.0,
                                    cP + cQ + 2.0 * cE, Alu.mult, Alu.add)
            ey2 = tnew("ey2")
            nc.vector.tensor_scalar(ey2[:], tS[:], 2.0,
                                    cP - cQ + 2.0 * cS, Alu.mult, Alu.add)
            xk = tnew("xk")
            eng_tt(Alu.add).tensor_tensor(xk[:], pq_s[:], ex2[:], Alu.add)
            yk = tnew("yk")
            eng_tt(Alu.add).tensor_tensor(yk[:], pq_d[:], ey2[:], Alu.add)
            x2 = tnew("x2")
            eng_tt(Alu.mult).tensor_tensor(x2[:], xk[:], xk[:], Alu.mult)
            y2 = tnew("y2")
            eng_tt(Alu.mult).tensor_tensor(y2[:], yk[:], yk[:], Alu.mult)
            mk = tnew("mk")
            eng_tt(Alu.add).tensor_tensor(mk[:], x2[:], y2[:], Alu.add)
            if mrun is None:
                mrun = mk
            else:
                m2 = tnew("mrun")
                eng_tt(Alu.max).tensor_tensor(m2[:], mrun[:], mk[:], Alu.max)
                mrun = m2
        nc.vector.tensor_single_scalar(m3[:, :, c0:c0 + CW], mrun[:],
                                       3.0, Alu.min)

    # ---------------- phase C: combine ----------------
    if "C" not in PH:
        acc0 = pool.tile([128, 1], FP32, tag="acc0", name="acc0")
        nc.vector.memset(acc0[:], 0.0)
        nc.sync.dma_start(out_partial.ap(), acc0[:])
        ctx.close()
        return
    if "A" not in PH:
        s_scr_init = pool.tile([128, 4096], FP32, tag="sinit",
                               name="sinit")
        nc.vector.memset(s_scr_init[:], 1.0)
        nc.sync.dma_start(
            bass.AP(tensor=s_scr, offset=0, ap=[[4096, 128], [1, 4096]]),
            s_scr_init[:])
        nc.sync.dma_start(
            bass.AP(tensor=s_scr, offset=4096 * 128, ap=[[1, 4]]),
            s_scr_init[0:1, 0:4])
    accs = []
    for h in range(2):
        ro = h * 2          # row-block offset (2 blocks per half)
        po = h * 2 * 65536  # pixel offset of this half
        s1t = pool.tile([128, 2, W], FP32, tag="s1t", bufs=2, name="s1t")
        nc.sync.dma_start(
            s1t[:], bass.AP(tensor=s_scr, offset=po,
                            ap=[[512, 128], [65536, 2], [1, W]]))
        s2t = pool.tile([128, 2, W], FP32, tag="s2t", bufs=2, name="s2t")
        nc.sync.dma_start(
            s2t[:], bass.AP(tensor=s_scr, offset=HWP + po,
                            ap=[[512, 128], [65536, 2], [1, W]]))
        nc.scalar.activation(s1t[:], s1t[:], Act.Ln)
        nc.scalar.activation(s2t[:], s2t[:], Act.Ln)
        u = s1t
        nc.vector.tensor_sub(u[:], s1t[:], s2t[:])

        # w = cubic(m3): Horner
        m3f = pool.tile([128, 2, W], FP32, tag="m3f", bufs=2, name="m3f")
        nc.vector.tensor_copy(m3f[:], m3[:, ro:ro + 2, :])
        h1 = pool.tile([128, 2, W], FP32, tag="h1", bufs=2, name="h1")
        nc.vector.tensor_scalar(h1[:], m3f[:], float(PC[3]), float(PC[2]),
                                Alu.mult, Alu.add)
        h2 = pool.tile([128, 2, W], FP32, tag="h2", bufs=2, name="h2")
        nc.vector.tensor_tensor(h2[:], h1[:], m3f[:], Alu.mult)
        nc.vector.tensor_scalar(h1[:], h2[:], 1.0, float(PC[1]),
                                Alu.mult, Alu.add)
        nc.vector.tensor_tensor(h2[:], h1[:], m3f[:], Alu.mult)
        nc.vector.tensor_scalar(h1[:], h2[:], 1.0, float(PC[0]),
                                Alu.mult, Alu.add)
        acc = pool.tile([128, 1], FP32, tag=f"acc{h}", name="acc")
        if "R" in PH:
            nc.vector.tensor_tensor(h2[:], h1[:], u[:], Alu.mult)
            nc.vector.tensor_reduce(acc[:],
                                    h2.rearrange("p a b -> p (a b)"),
                                    mybir.AxisListType.X, Alu.add)
        else:
            nc.vector.memset(acc[:], 0.0)
        accs.append(acc)
    acc = pool.tile([128, 1], FP32, tag="accT", name="accT")
    nc.vector.tensor_tensor(acc[:], accs[0][:], accs[1][:], Alu.add)
    nc.sync.dma_start(out_partial.ap(), acc[:])
    ctx.close()


_NC_CACHE = None


def _get_nc():
    global _NC_CACHE
    if _NC_CACHE is None:
        _NC_CACHE = build_nc()
    return _NC_CACHE


def kernel(logits, targets):
    logits = np.ascontiguousarray(np.asarray(logits, dtype=np.float32))
    targets = np.ascontiguousarray(np.asarray(targets)).astype(np.int32)
    assert logits.shape == (B, C, H, W), logits.shape
    assert targets.shape == (B, H, W), targets.shape

    nc = _get_nc()
    cmaps = host_consts()
    pad = np.zeros(PADIN, np.float32)
    in_maps = [
        {"logits": np.concatenate([logits[b].reshape(-1), pad]),
         "targets": targets[b].copy(), **cmaps}
        for b in range(NCORES)
    ]
    res = run_bass_kernel_spmd(nc, in_maps, list(range(NCORES)))
    total = 0.0
    for r in res.results:
        total += float(np.asarray(r["out_partial"], np.float64).sum())
    return np.float32(total / (B * H * W))

